# revision 32
# baseline (speedup 1.0000x reference)
"""GCN-sampling (NodeFlow) kernel for 8 Trainium2 NeuronCores.

Strategy (single NEFF, SPMD by data, no collectives):
  - features packed fp8_e3m4 into [N0, 512B] rows, typed uint64[N0, 64]
    (gather/scatter are byte movers; u64 typing minimizes element count).
  - Layer-1 nodes (N1=25000) sharded 8-way (3125/core), split into 5
    groups of 5 node-superblocks (640 nodes each).
  - Stage A per (group, window) cell: dma_gather the cell's neighbor rows
    (row ids sorted by destination position, int16 window-local ids), then
    dma_scatter_add each row into a zeroed per-group DRAM staging table C_g
    at its node-sorted position (16*node_local + k). Zero + add == write.
  - Stage B per superblock: iota-gather 2048 node-aligned positions from
    C_g, then 16 blocks x 4 feature-chunk matmuls with a CONSTANT block-sum
    lhsT/rhs (S[i, i//16]=1): psum m0T [128 feat, 4, 128 nodes], each
    (block, chunk) single-shot (a node's 16 rows live in one block).
  - Epilogue per sb: m0T -> f16, 4 W1 matmuls -> h1 [h, nodes] psum, bias +
    identity/relu (ACT, per-partition bias), then qp = [nodes, 47] directly
    via lhsT=h1cat half / rhs=W2 half (no transposes anywhere), f16 Q rows
    written into one of two DRAM Q-tables (split so stage 2 starts early).
  - Stage 2: refs bucketed by (Q-half, seed-group); seed-sorted u64 gathers
    of 256B Q rows; one-hot S matrices precomputed on host (fp8, DMA'd) as
    matmul lhsT; per (half, seed-sb) partial [128, 47] psum accumulated
    across blocks, written to partial[half]. Host sums 8 cores x 2 halves
    and adds b2.
All matmul PSUM f32; 1/16 mean factors folded into W1/W2 on host.
"""

import sys

sys.path.insert(0, "/opt/trn_rl_repo")

import hashlib

import numpy as np
import ml_dtypes

import concourse.bass as bass
import concourse.mybir as mybir
from concourse import bacc
from concourse.tile import TileContext
from concourse.bass_utils import run_bass_kernel_spmd

N0, N1, N2 = 200000, 25000, 5000
FANOUT = 16
IN_F, NH, NCLS = 500, 128, 47
NCORES = 8
WINDOW = 28572  # ceil(N0/7) <= 32767 so window-local row ids fit int16
NWIN = (N0 + WINDOW - 1) // WINDOW  # 7
E1 = 512  # fp8 bytes per feature row
E1U = E1 // 8  # 64 u64 elements
E2U = 32  # u64 elements per Q row (256B = 128 f16)
NODES_PER_CORE = N1 // NCORES  # 3125
NSB1 = (NODES_PER_CORE + 127) // 128  # 25 (sb24 has 53 real nodes)
GROUP1 = 5  # sbs per group
NG1 = NSB1 // GROUP1  # 5
NPG = GROUP1 * 128  # nodes per group (640)
POSG = NPG * FANOUT  # positions per group (10240)
TRASH = POSG  # scatter position for cap-padding refs
CROWS = POSG + 128  # staging table rows per group
SEEDS = N2
NSB2 = (SEEDS + 127) // 128  # 40 seed sbs
SG2 = 8  # seed-groups (5 sbs each)
QPART_GROUPS = [(0, 2), (2, 5)]  # node-group ranges per Q part
NPARTS = len(QPART_GROUPS)
QPART_ROWS = [(b - a) * NPG for a, b in QPART_GROUPS]
QPART_BASE = [0, QPART_ROWS[0], QPART_ROWS[0] + QPART_ROWS[1]]
MAXIDX = 1024

f16 = mybir.dt.float16
f32 = mybir.dt.float32
f8 = mybir.dt.float8e3
i16 = mybir.dt.int16
u64 = mybir.dt.uint64
u32 = mybir.dt.uint32


def _wrap_idxs(flat):
    """[n] -> [128, n/16] int16: index i at [i%16, i//16], replicated x8."""
    n = len(flat)
    assert n % 16 == 0
    a = np.empty((128, n // 16), np.int16)
    blk = flat.reshape(n // 16, 16).T
    for g in range(8):
        a[g * 16 : (g + 1) * 16, :] = blk
    return a


def _rup(x, m):
    return (x + m - 1) // m * m


def _assign_nodes(src0):
    """node -> (core, local slot); identity block assignment."""
    n = np.arange(N1)
    return n // NODES_PER_CORE, n % NODES_PER_CORE


def _plan_stage1(src0, core_of, loc_of):
    """Per-core stage-A gather/scatter streams + shared cell caps.

    Returns (caps [NG1*NWIN], percore [(idx_i16, pos_i16)]).
    """
    percell = []  # [core][cell] -> (win_row, pos)
    counts = np.zeros((NCORES, NG1 * NWIN), np.int64)
    allrows = np.asarray(src0)
    for c in range(NCORES):
        mine = np.nonzero(core_of == c)[0]
        s = allrows[mine]
        nl = np.repeat(loc_of[mine], FANOUT)
        k = np.tile(np.arange(FANOUT), len(mine))
        row = s.reshape(-1)
        g = nl // NPG
        w = row // WINDOW
        cell = g * NWIN + w
        pos = (nl - g * NPG) * FANOUT + k
        order = np.lexsort((pos, cell))
        row, cell, pos, w = row[order], cell[order], pos[order], w[order]
        starts = np.searchsorted(cell, np.arange(NG1 * NWIN))
        ends = np.searchsorted(cell, np.arange(NG1 * NWIN), side="right")
        counts[c] = ends - starts
        percell.append(
            [
                ((row - w * WINDOW)[starts[i] : ends[i]], pos[starts[i] : ends[i]])
                for i in range(NG1 * NWIN)
            ]
        )
    caps = np.array([_rup(m, 16) for m in counts.max(axis=0)], np.int64)
    percore = []
    for c in range(NCORES):
        idxs = []
        poss = []
        for i in range(NG1 * NWIN):
            r, p = percell[c][i]
            cap = int(caps[i])
            ri = np.zeros(cap, np.int16)
            pi = np.full(cap, TRASH, np.int16)
            ri[: len(r)] = r.astype(np.int16)
            if len(r) < cap:
                ri[len(r) :] = r[-1] if len(r) else 0
            pi[: len(p)] = p.astype(np.int16)
            idxs.append(ri)
            poss.append(pi)
        percore.append((np.concatenate(idxs), np.concatenate(poss)))
    return caps, percore


def _plan_stage2(src1, core_of, loc_of):
    """Stage-2: cells = (qhalf, seedgroup); shared union schedule + per-core
    idx stream and host-built one-hot S columns (fp8)."""
    NCELL = NPARTS * SG2
    flat0 = np.asarray(src1).reshape(-1)
    seed0 = np.repeat(np.arange(SEEDS), FANOUT)
    keys = []
    cell_of = []
    for c in range(NCORES):
        m = core_of[flat0] == c
        local, seed = loc_of[flat0[m]], seed0[m]
        grp = local // NPG
        qh = np.searchsorted(np.array([b for _, b in QPART_GROUPS]), grp, side="right")
        rowloc = local - np.array(QPART_BASE)[qh] * 1
        sb = seed // 128
        cell = qh * SG2 + seed // (128 * 5)
        psb = qh * NSB2 + sb
        order = np.lexsort((rowloc, psb, cell))
        keys.append((rowloc[order], psb[order], (seed - sb * 128)[order]))
        cell_of.append(cell[order])

    counts = np.zeros((NCORES, NCELL), np.int64)
    for c in range(NCORES):
        counts[c] = np.bincount(cell_of[c], minlength=NCELL)
    # 128-mult so every gathered block is fully written (matmul rhs reads
    # whole blocks; SBUF garbage partitions would poison psum via 0*NaN)
    caps = np.array([_rup(m, 128) for m in counts.max(axis=0)], np.int64)

    percell = []
    for c in range(NCORES):
        rowloc, psb, slot = keys[c]
        co = cell_of[c]
        starts = np.searchsorted(co, np.arange(NCELL))
        ends = np.searchsorted(co, np.arange(NCELL), side="right")
        percell.append(
            [
                (rowloc[starts[i] : ends[i]], psb[starts[i] : ends[i]], slot[starts[i] : ends[i]])
                for i in range(NCELL)
            ]
        )

    # union matmul schedule per cell
    schedule = []
    for i in range(NCELL):
        cap = int(caps[i])
        ms = []
        for b in range(_rup(cap, 128) // 128):
            present = set()
            for c in range(NCORES):
                sbv = percell[c][i][1][b * 128 : (b + 1) * 128]
                present.update(np.unique(sbv).tolist())
            for s in sorted(present):
                ms.append((b, int(s)))
        # psb-major so psum accumulation groups don't interleave within the
        # shared per-cell psum tile
        ms.sort(key=lambda t: (t[1], t[0]))
        schedule.append(ms)
    ncol = sum(len(s) for s in schedule)

    percore = []
    for c in range(NCORES):
        idx_all = []
        scols = np.zeros((128, ncol * 128), ml_dtypes.float8_e3m4)
        mcol = 0
        for i in range(NCELL):
            cap = int(caps[i])
            rowloc, psbv, slot = percell[c][i]
            n = len(rowloc)
            ri = np.zeros(cap, np.int16)
            ri[:n] = rowloc.astype(np.int16)
            if n < cap:
                ri[n:] = rowloc[-1] if n else 0
            idx_all.append(ri)
            pa = np.full(_rup(cap, 128), -999, np.int64)
            sa = np.full(_rup(cap, 128), -1, np.int64)
            pa[:n] = psbv
            sa[:n] = slot
            for b, ps in schedule[i]:
                m = pa[b * 128 : (b + 1) * 128] == ps
                rows = np.nonzero(m)[0]
                cols = sa[b * 128 : (b + 1) * 128][m]
                scols[rows, mcol * 128 + cols] = 1.0
                mcol += 1
        percore.append((np.concatenate(idx_all), scols))
    return caps, schedule, percore


def build_kernel(plan1, plan2):
    caps1, _ = plan1
    caps2, sched2, _ = plan2
    nc = bacc.Bacc(None, target_bir_lowering=False, debug=False)

    tot1 = int(caps1.sum())
    tot2 = int(caps2.sum())
    ncol2 = sum(len(s) for s in sched2)

    sbtot2 = np.zeros(NPARTS * NSB2, np.int64)
    for s in sched2:
        for _, ps in s:
            sbtot2[ps] += 1

    ftab = nc.dram_tensor("ftab", [N0, E1U], u64, kind="ExternalInput")
    idx1 = nc.dram_tensor("idx1", [128, tot1 // 16], i16, kind="ExternalInput")
    pos1 = nc.dram_tensor("pos1", [128, tot1 // 16], i16, kind="ExternalInput")
    iotab = nc.dram_tensor("iotab", [128, MAXIDX // 16], i16, kind="ExternalInput")
    idx2 = nc.dram_tensor("idx2", [128, tot2 // 16], i16, kind="ExternalInput")
    s2m = nc.dram_tensor("s2m", [128, ncol2 * 128], f8, kind="ExternalInput")
    ctabs = [
        nc.dram_tensor(f"ctab{g}", [CROWS, E1U], u64, kind="ExternalInput")
        for g in range(NG1)
    ]
    w1t = nc.dram_tensor("w1t", [128, 4, NH], f16, kind="ExternalInput")  # W1/16
    b1v = nc.dram_tensor("b1v", [128, 1], f32, kind="ExternalInput")
    w2t = nc.dram_tensor("w2t", [128, 2, NCLS], f16, kind="ExternalInput")  # W2/16
    scon = nc.dram_tensor("scon", [128, 8], f16, kind="ExternalInput")
    partial = nc.dram_tensor(
        "partial", [NPARTS, SG2, 128, 5 * NCLS], f32, kind="ExternalOutput"
    )

    # stage-1 per-group idx stream column offsets (in refs)
    cell_off = np.zeros(NG1 * NWIN + 1, np.int64)
    np.cumsum(caps1, out=cell_off[1:])

    with TileContext(nc) as tc:
        with (
            tc.tile_pool(name="const", bufs=1) as cpool,
            tc.tile_pool(name="ga", bufs=2) as gapool,
            tc.tile_pool(name="gb", bufs=4) as gbpool,
            tc.tile_pool(name="g2", bufs=6) as g2pool,
            tc.tile_pool(name="s2", bufs=6) as s2pool,
            tc.tile_pool(name="epi", bufs=3) as epool,
            tc.tile_pool(name="m0psum", bufs=2, space="PSUM") as mpool,
            tc.tile_pool(name="epipsum", bufs=3, space="PSUM") as eppool,
            tc.tile_pool(name="dram", bufs=1, space="DRAM") as dpool,
        ):
            # per-group idx/pos loads so the first gather starts early
            idx1_t = cpool.tile([128, tot1 // 16], i16)
            pos1_t = cpool.tile([128, tot1 // 16], i16)
            c00 = int(cell_off[1]) // 16
            nc.sync.dma_start(idx1_t[:, :c00], idx1[:, :c00])
            for g in range(NG1):
                a = int(cell_off[g * NWIN]) // 16
                b = int(cell_off[(g + 1) * NWIN]) // 16
                if g == 0:
                    nc.sync.dma_start(idx1_t[:, c00:b], idx1[:, c00:b])
                else:
                    nc.sync.dma_start(idx1_t[:, a:b], idx1[:, a:b])
                nc.sync.dma_start(pos1_t[:, a:b], pos1[:, a:b])
            iotab_t = cpool.tile([128, MAXIDX // 16], i16)
            nc.sync.dma_start(iotab_t[:], iotab[:])
            idx2_t = cpool.tile([128, tot2 // 16], i16)
            nc.sync.dma_start(idx2_t[:], idx2[:])
            w1_t = cpool.tile([128, 4, NH], f16)
            nc.sync.dma_start(w1_t[:], w1t[:])
            b1_t = cpool.tile([128, 1], f32)
            nc.sync.dma_start(b1_t[:], b1v[:])
            w2_t = cpool.tile([128, 2, NCLS], f16)
            nc.sync.dma_start(w2_t[:], w2t[:])
            scon_t = cpool.tile([128, 8], f16)
            nc.sync.dma_start(scon_t[:], scon[:])

            qtabs = [dpool.tile([QPART_ROWS[p], E2U], u64, name=f"qtab{p}")
                     for p in range(NPARTS)]

            s2_tiles = []
            off2 = 0
            for i in range(NPARTS * SG2):
                ncell = len(sched2[i])
                if ncell:
                    t = cpool.tile([128, ncell, 128], f8, name=f"s_t{i}")
                    nc.sync.dma_start(t[:], s2m[:, off2 * 128 : (off2 + ncell) * 128])
                    s2_tiles.append(t)
                else:
                    s2_tiles.append(None)
                off2 += ncell

            # ---- stage 1 ----
            def do_gathers(g):
                tiles = []
                for w in range(NWIN):
                    cell = g * NWIN + w
                    n = int(caps1[cell])
                    if n == 0:
                        tiles.append(None)
                        continue
                    nb = _rup(n, 128) // 128
                    g_t = gapool.tile([128, nb, E1U], u64, tag="ga", name=f"g_t{w}")
                    wsz = min(WINDOW, N0 - w * WINDOW)
                    base = cell_off[cell]
                    off = 0
                    while off < n:
                        m = min(MAXIDX, n - off)
                        mb = _rup(m, 128) // 128
                        nc.gpsimd.dma_gather(
                            out_ap=g_t[:, off // 128 : off // 128 + mb, :],
                            in_ap=ftab[w * WINDOW : w * WINDOW + wsz, :],
                            idxs_ap=idx1_t[:, (base + off) // 16 : (base + off + m) // 16],
                            num_idxs=m,
                            num_idxs_reg=m,
                            elem_size=E1U,
                        )
                        off += m
                    tiles.append(g_t)
                return tiles

            def do_scatters(g, tiles):
                ctab = ctabs[g]
                for w in range(NWIN):
                    cell = g * NWIN + w
                    n = int(caps1[cell])
                    if n == 0:
                        continue
                    g_t = tiles[w]
                    base = cell_off[cell]
                    off = 0
                    while off < n:
                        m = min(MAXIDX, n - off)
                        mb = _rup(m, 128) // 128
                        nc.gpsimd.dma_scatter_add(
                            out_ap=ctab[:],
                            in_ap=g_t[:, off // 128 : off // 128 + mb, :],
                            idxs_ap=pos1_t[:, (base + off) // 16 : (base + off + m) // 16],
                            num_idxs=m,
                            num_idxs_reg=m,
                            elem_size=E1U,
                        )
                        off += m

            def do_compute(g):
                ctab = ctabs[g]
                for j in range(GROUP1):
                    sb = g * GROUP1 + j
                    b_t = gbpool.tile([128, 16, E1U], u64, tag="gb", name="b_t")
                    for h in range(2):
                        nc.gpsimd.dma_gather(
                            out_ap=b_t[:, h * 8 : h * 8 + 8, :],
                            in_ap=ctab[j * 2048 + h * 1024 : j * 2048 + h * 1024 + 1024, :],
                            idxs_ap=iotab_t[:],
                            num_idxs=MAXIDX,
                            num_idxs_reg=MAXIDX,
                            elem_size=E1U,
                        )
                    m0t = mpool.tile([128, 4, 128], f32, tag="m0", name="m0t")
                    bf8 = b_t[:].bitcast(f8)  # [128, 16, 512]
                    for b in range(16):
                        for k in range(4):
                            nc.tensor.matmul(
                                out=m0t[:, k, 8 * b : 8 * b + 8],
                                lhsT=bf8[:, b, k * 128 : (k + 1) * 128],
                                rhs=scon_t[:],
                                start=True,
                                stop=True,
                            )
                    # epilogue
                    m0s = epool.tile([128, 4, 128], f16, tag="m0s", name="m0s")
                    nc.vector.tensor_copy(m0s[:], m0t[:])
                    h1p = eppool.tile([128, 128], f32, tag="ep", name="h1p")
                    for k in range(4):
                        nc.tensor.matmul(
                            out=h1p[:],
                            lhsT=w1_t[:, k, :],
                            rhs=m0s[:, k, :],
                            start=(k == 0),
                            stop=(k == 3),
                        )
                    h1s = epool.tile([128, 128], f16, tag="h1", name="h1s")
                    rs = epool.tile([128, 128], f16, tag="r", name="rs")
                    nc.scalar.activation(
                        h1s[:], h1p[:], mybir.ActivationFunctionType.Identity, bias=b1_t[:, :1]
                    )
                    nc.scalar.activation(
                        rs[:], h1p[:], mybir.ActivationFunctionType.Relu, bias=b1_t[:, :1]
                    )
                    qp = eppool.tile([128, NCLS], f32, tag="ep", name="qp")
                    nc.tensor.matmul(out=qp[:], lhsT=h1s[:], rhs=w2_t[:, 0, :], start=True, stop=False)
                    nc.tensor.matmul(out=qp[:], lhsT=rs[:], rhs=w2_t[:, 1, :], start=False, stop=True)
                    qs = epool.tile([128, NCLS], f16, tag="qs", name="qs")
                    nc.scalar.activation(qs[:], qp[:], mybir.ActivationFunctionType.Copy)
                    part = next(p for p, (a2, b2) in enumerate(QPART_GROUPS) if a2 <= g < b2)
                    r0 = sb * 128 - QPART_BASE[part]
                    dst = qtabs[part][r0 : r0 + 128, :]
                    nc.sync.dma_start(dst.bitcast(f16)[:, :NCLS], qs[:])

            # ---- stage 2 ----
            cell2_off = np.zeros(NPARTS * SG2 + 1, np.int64)
            np.cumsum(caps2, out=cell2_off[1:])
            sb2_seq = np.zeros(NPARTS * NSB2, np.int64)
            pp = {}
            mcol = [0]

            def do_stage2_cell(i):
                qh = i // SG2
                qt = qtabs[qh]
                n = int(caps2[i])
                pgrp = epool.tile([128, 5, NCLS], f32, tag="pg", name="pg", bufs=4)
                cellpp = eppool.tile([128, 5, NCLS], f32, tag="ep", name="cellpp")
                written = set()
                if n:
                    nb = _rup(n, 128) // 128
                    g_t = g2pool.tile([128, nb, E2U], u64, tag="g2", name="g_t2")
                    base = cell2_off[i]
                    off = 0
                    while off < n:
                        m = min(MAXIDX, n - off)
                        mb = _rup(m, 128) // 128
                        nc.gpsimd.dma_gather(
                            out_ap=g_t[:, off // 128 : off // 128 + mb, :],
                            in_ap=qt[:],
                            idxs_ap=idx2_t[:, (base + off) // 16 : (base + off + m) // 16],
                            num_idxs=m,
                            num_idxs_reg=m,
                            elem_size=E2U,
                        )
                        off += m
                    gf16 = g_t[:].bitcast(f16)  # [128, nb, 128]
                    ncell = len(sched2[i])
                    s_t = s2_tiles[i]
                    for mi, (b, ps) in enumerate(sched2[i]):
                        jj = (ps % NSB2) % 5
                        nc.tensor.matmul(
                            out=cellpp[:, jj, :],
                            lhsT=s_t[:, mi, :],
                            rhs=gf16[:, b, :NCLS],
                            start=(sb2_seq[ps] == 0),
                            stop=(sb2_seq[ps] == sbtot2[ps] - 1),
                        )
                        sb2_seq[ps] += 1
                        if sb2_seq[ps] == sbtot2[ps]:
                            written.add(jj)
                    mcol[0] += ncell
                for jj in range(5):
                    if jj not in written:
                        nc.vector.memset(cellpp[:, jj, :], 0.0)
                if i % 2 == 0:
                    nc.vector.tensor_copy(pgrp[:], cellpp[:])
                else:
                    nc.scalar.activation(
                        pgrp[:], cellpp[:], mybir.ActivationFunctionType.Copy
                    )
                sg = i % SG2
                eng = nc.sync if i % 2 == 0 else nc.scalar
                eng.dma_start(
                    partial[qh, sg, :, :].rearrange("p (j e) -> p j e", j=5),
                    pgrp[:, :, :],
                )

            # software pipeline: gathers(g) | compute(g-1) | scatters(g);
            # stage-2 parts go last, in readiness order, so no Pool
            # instruction queues behind a long epilogue dependency
            tiles = do_gathers(0)
            do_scatters(0, tiles)
            for g in range(1, NG1):
                tiles = do_gathers(g)
                do_compute(g - 1)
                do_scatters(g, tiles)
                if g - 1 == 2:
                    for i in range(SG2):
                        do_stage2_cell(i)
            do_compute(NG1 - 1)
            for i in range(SG2, NPARTS * SG2):
                do_stage2_cell(i)
    nc.compile()
    return nc


def _host_inputs(features, src0, src1, W1, b1, W2):
    core_of, loc_of = _assign_nodes(src0)
    plan1 = _plan_stage1(src0, core_of, loc_of)
    plan2 = _plan_stage2(src1, core_of, loc_of)

    ftab_f8 = np.zeros((N0, E1), ml_dtypes.float8_e3m4)
    ftab_f8[:, :IN_F] = np.asarray(features, np.float32).astype(ml_dtypes.float8_e3m4)
    ftab_u64 = np.ascontiguousarray(ftab_f8).view(np.uint64)

    w1_np = np.zeros((128, 4, NH), np.float16)
    w1f = np.zeros((E1, NH), np.float32)
    w1f[:IN_F] = np.asarray(W1, np.float32) / FANOUT
    for k in range(4):
        w1_np[:, k, :] = w1f[k * 128 : (k + 1) * 128].astype(np.float16)
    b1_np = np.asarray(b1, np.float32).reshape(128, 1)
    w2_np = np.zeros((128, 2, NCLS), np.float16)
    w2f = np.asarray(W2, np.float32) / FANOUT
    w2_np[:, 0, :] = w2f[:NH].astype(np.float16)
    w2_np[:, 1, :] = w2f[NH:].astype(np.float16)
    scon_np = np.zeros((128, 8), np.float16)
    scon_np[np.arange(128), np.arange(128) // 16] = 1.0
    iotab_np = _wrap_idxs(np.arange(MAXIDX, dtype=np.int16))
    czero = np.zeros((CROWS, E1U), np.uint64)

    in_maps = []
    for c in range(NCORES):
        idx1c, pos1c = plan1[1][c]
        idx2c, s2c = plan2[2][c]
        im = {
            "ftab": ftab_u64,
            "idx1": np.ascontiguousarray(_wrap_idxs(idx1c)),
            "pos1": np.ascontiguousarray(_wrap_idxs(pos1c)),
            "iotab": iotab_np,
            "idx2": np.ascontiguousarray(_wrap_idxs(idx2c)),
            "s2m": np.ascontiguousarray(s2c),
            "w1t": w1_np,
            "b1v": b1_np,
            "w2t": w2_np,
            "scon": scon_np,
        }
        for g in range(NG1):
            im[f"ctab{g}"] = czero
        in_maps.append(im)
    return plan1, plan2, in_maps


_cache = {}


def kernel(features, src0, src1, W1, b1, W2, b2):
    plan1, plan2, in_maps = _host_inputs(features, src0, src1, W1, b1, W2)
    key = hashlib.sha256(
        b"|".join(
            [plan1[0].tobytes(), plan2[0].tobytes(), str(plan2[1]).encode(), b"v2"]
        )
    ).hexdigest()
    if key not in _cache:
        _cache[key] = build_kernel(plan1, plan2)
    nc = _cache[key]
    res = run_bass_kernel_spmd(nc, in_maps, core_ids=list(range(NCORES)))
    out = np.zeros((SEEDS, NCLS), np.float64)
    for c in range(NCORES):
        p = res.results[c]["partial"].astype(np.float64)
        p = p.reshape(NPARTS, SG2, 128, 5, NCLS).transpose(0, 1, 3, 2, 4)
        out += p.reshape(NPARTS, SG2 * 640, NCLS)[:, :SEEDS].sum(axis=0)
    out = out + np.asarray(b2, np.float64)[None, :]
    return out.astype(np.float32)


if __name__ == "__main__":
    rng = np.random.default_rng(0)
    feats = rng.standard_normal((N0, IN_F), dtype=np.float32)
    src0 = rng.integers(0, N0, size=(N1, FANOUT))
    src1 = rng.integers(0, N1, size=(N2, FANOUT))
    W1 = rng.standard_normal((IN_F, NH), dtype=np.float32) * 0.05
    b1 = np.zeros(NH, np.float32)
    W2 = rng.standard_normal((2 * NH, NCLS), dtype=np.float32) * 0.05
    b2 = np.zeros(NCLS, np.float32)
    out = kernel(feats, src0, src1, W1, b1, W2, b2)
    m0 = feats[src0].mean(axis=1)
    h1 = m0 @ W1 + b1
    h1 = np.concatenate([h1, np.maximum(h1, 0)], axis=1)
    m1 = h1[src1].mean(axis=1)
    ref = m1 @ W2 + b2
    rel = np.abs(out - ref) / (np.abs(ref) + 1e-5)
    print("max rel err:", rel.max(), "mean:", rel.mean())
    print("norm rel:", np.linalg.norm(out - ref) / np.linalg.norm(ref))


# revision 33
# speedup vs baseline: 1.0199x; 1.0199x over previous
"""GCN-sampling (NodeFlow) kernel for 8 Trainium2 NeuronCores.

Strategy (single NEFF, SPMD by data, no collectives):
  - features packed fp8_e3m4 into [N0, 512B] rows, typed uint64[N0, 64]
    (gather/scatter are byte movers; u64 typing minimizes element count).
  - Layer-1 nodes (N1=25000) sharded 8-way (3125/core), split into 5
    groups of 5 node-superblocks (640 nodes each).
  - Stage A per (group, window) cell: dma_gather the cell's neighbor rows
    (row ids sorted by destination position, int16 window-local ids), then
    dma_scatter_add each row into a zeroed per-group DRAM staging table C_g
    at its node-sorted position (16*node_local + k). Zero + add == write.
  - Stage B per superblock: iota-gather 2048 node-aligned positions from
    C_g, then 16 blocks x 4 feature-chunk matmuls with a CONSTANT block-sum
    lhsT/rhs (S[i, i//16]=1): psum m0T [128 feat, 4, 128 nodes], each
    (block, chunk) single-shot (a node's 16 rows live in one block).
  - Epilogue per sb: m0T -> f16, 4 W1 matmuls -> h1 [h, nodes] psum, bias +
    identity/relu (ACT, per-partition bias), then qp = [nodes, 47] directly
    via lhsT=h1cat half / rhs=W2 half (no transposes anywhere), f16 Q rows
    written into one of two DRAM Q-tables (split so stage 2 starts early).
  - Stage 2: refs bucketed by (Q-half, seed-group); seed-sorted u64 gathers
    of 256B Q rows; one-hot S matrices precomputed on host (fp8, DMA'd) as
    matmul lhsT; per (half, seed-sb) partial [128, 47] psum accumulated
    across blocks, written to partial[half]. Host sums 8 cores x 2 halves
    and adds b2.
All matmul PSUM f32; 1/16 mean factors folded into W1/W2 on host.
"""

import sys

sys.path.insert(0, "/opt/trn_rl_repo")

import hashlib

import numpy as np
import ml_dtypes

import concourse.bass as bass
import concourse.mybir as mybir
from concourse import bacc
from concourse.tile import TileContext
from concourse.bass_utils import run_bass_kernel_spmd

N0, N1, N2 = 200000, 25000, 5000
FANOUT = 16
IN_F, NH, NCLS = 500, 128, 47
NCORES = 8
WINDOW = 28572  # ceil(N0/7) <= 32767 so window-local row ids fit int16
NWIN = (N0 + WINDOW - 1) // WINDOW  # 7
E1 = 512  # fp8 bytes per feature row
E1U = E1 // 8  # 64 u64 elements
E2U = 32  # u64 elements per Q row (256B = 128 f16)
NODES_PER_CORE = N1 // NCORES  # 3125
NSB1 = (NODES_PER_CORE + 127) // 128  # 25 (sb24 has 53 real nodes)
GROUP1 = 5  # sbs per group
NG1 = NSB1 // GROUP1  # 5
NPG = GROUP1 * 128  # nodes per group (640)
POSG = NPG * FANOUT  # positions per group (10240)
TRASH = POSG  # scatter position for cap-padding refs
CROWS = POSG + 128  # staging table rows per group
SEEDS = N2
NSB2 = (SEEDS + 127) // 128  # 40 seed sbs
SG2 = 8  # seed-groups (5 sbs each)
QPART_GROUPS = [(0, 2), (2, 5)]  # node-group ranges per Q part
NPARTS = len(QPART_GROUPS)
QPART_ROWS = [(b - a) * NPG for a, b in QPART_GROUPS]
QPART_BASE = [0, QPART_ROWS[0], QPART_ROWS[0] + QPART_ROWS[1]]
MAXIDX = 1024

f16 = mybir.dt.float16
f32 = mybir.dt.float32
f8 = mybir.dt.float8e3
i16 = mybir.dt.int16
u64 = mybir.dt.uint64
u32 = mybir.dt.uint32


def _wrap_idxs(flat):
    """[n] -> [128, n/16] int16: index i at [i%16, i//16], replicated x8."""
    n = len(flat)
    assert n % 16 == 0
    a = np.empty((128, n // 16), np.int16)
    blk = flat.reshape(n // 16, 16).T
    for g in range(8):
        a[g * 16 : (g + 1) * 16, :] = blk
    return a


def _rup(x, m):
    return (x + m - 1) // m * m


def _assign_nodes(src0):
    """node -> (core, local slot); identity block assignment."""
    n = np.arange(N1)
    return n // NODES_PER_CORE, n % NODES_PER_CORE


def _plan_stage1(src0, core_of, loc_of):
    """Per-core stage-A gather/scatter streams + shared cell caps.

    Returns (caps [NG1*NWIN], percore [(idx_i16, pos_i16)]).
    """
    percell = []  # [core][cell] -> (win_row, pos)
    counts = np.zeros((NCORES, NG1 * NWIN), np.int64)
    allrows = np.asarray(src0)
    for c in range(NCORES):
        mine = np.nonzero(core_of == c)[0]
        s = allrows[mine]
        nl = np.repeat(loc_of[mine], FANOUT)
        k = np.tile(np.arange(FANOUT), len(mine))
        row = s.reshape(-1)
        g = nl // NPG
        w = row // WINDOW
        cell = g * NWIN + w
        pos = (nl - g * NPG) * FANOUT + k
        order = np.lexsort((pos, cell))
        row, cell, pos, w = row[order], cell[order], pos[order], w[order]
        starts = np.searchsorted(cell, np.arange(NG1 * NWIN))
        ends = np.searchsorted(cell, np.arange(NG1 * NWIN), side="right")
        counts[c] = ends - starts
        percell.append(
            [
                ((row - w * WINDOW)[starts[i] : ends[i]], pos[starts[i] : ends[i]])
                for i in range(NG1 * NWIN)
            ]
        )
    caps = np.array([_rup(m, 16) for m in counts.max(axis=0)], np.int64)
    percore = []
    for c in range(NCORES):
        idxs = []
        poss = []
        for i in range(NG1 * NWIN):
            r, p = percell[c][i]
            cap = int(caps[i])
            ri = np.zeros(cap, np.int16)
            pi = np.full(cap, TRASH, np.int16)
            ri[: len(r)] = r.astype(np.int16)
            if len(r) < cap:
                ri[len(r) :] = r[-1] if len(r) else 0
            pi[: len(p)] = p.astype(np.int16)
            idxs.append(ri)
            poss.append(pi)
        percore.append((np.concatenate(idxs), np.concatenate(poss)))
    return caps, percore


def _plan_stage2(src1, core_of, loc_of):
    """Stage-2: cells = (qhalf, seedgroup); shared union schedule + per-core
    idx stream and host-built one-hot S columns (fp8)."""
    NCELL = NPARTS * SG2
    flat0 = np.asarray(src1).reshape(-1)
    seed0 = np.repeat(np.arange(SEEDS), FANOUT)
    keys = []
    cell_of = []
    for c in range(NCORES):
        m = core_of[flat0] == c
        local, seed = loc_of[flat0[m]], seed0[m]
        grp = local // NPG
        qh = np.searchsorted(np.array([b for _, b in QPART_GROUPS]), grp, side="right")
        rowloc = local - np.array(QPART_BASE)[qh] * 1
        sb = seed // 128
        cell = qh * SG2 + seed // (128 * 5)
        psb = qh * NSB2 + sb
        order = np.lexsort((rowloc, psb, cell))
        keys.append((rowloc[order], psb[order], (seed - sb * 128)[order]))
        cell_of.append(cell[order])

    counts = np.zeros((NCORES, NCELL), np.int64)
    for c in range(NCORES):
        counts[c] = np.bincount(cell_of[c], minlength=NCELL)
    # 128-mult so every gathered block is fully written (matmul rhs reads
    # whole blocks; SBUF garbage partitions would poison psum via 0*NaN)
    caps = np.array([_rup(m, 128) for m in counts.max(axis=0)], np.int64)

    percell = []
    for c in range(NCORES):
        rowloc, psb, slot = keys[c]
        co = cell_of[c]
        starts = np.searchsorted(co, np.arange(NCELL))
        ends = np.searchsorted(co, np.arange(NCELL), side="right")
        percell.append(
            [
                (rowloc[starts[i] : ends[i]], psb[starts[i] : ends[i]], slot[starts[i] : ends[i]])
                for i in range(NCELL)
            ]
        )

    # union matmul schedule per cell
    schedule = []
    for i in range(NCELL):
        cap = int(caps[i])
        ms = []
        for b in range(_rup(cap, 128) // 128):
            present = set()
            for c in range(NCORES):
                sbv = percell[c][i][1][b * 128 : (b + 1) * 128]
                present.update(np.unique(sbv).tolist())
            for s in sorted(present):
                ms.append((b, int(s)))
        # psb-major so psum accumulation groups don't interleave within the
        # shared per-cell psum tile
        ms.sort(key=lambda t: (t[1], t[0]))
        schedule.append(ms)
    ncol = sum(len(s) for s in schedule)

    percore = []
    for c in range(NCORES):
        idx_all = []
        scols = np.zeros((128, ncol * 128), ml_dtypes.float8_e3m4)
        mcol = 0
        for i in range(NCELL):
            cap = int(caps[i])
            rowloc, psbv, slot = percell[c][i]
            n = len(rowloc)
            ri = np.zeros(cap, np.int16)
            ri[:n] = rowloc.astype(np.int16)
            if n < cap:
                ri[n:] = rowloc[-1] if n else 0
            idx_all.append(ri)
            pa = np.full(_rup(cap, 128), -999, np.int64)
            sa = np.full(_rup(cap, 128), -1, np.int64)
            pa[:n] = psbv
            sa[:n] = slot
            for b, ps in schedule[i]:
                m = pa[b * 128 : (b + 1) * 128] == ps
                rows = np.nonzero(m)[0]
                cols = sa[b * 128 : (b + 1) * 128][m]
                scols[rows, mcol * 128 + cols] = 1.0
                mcol += 1
        percore.append((np.concatenate(idx_all), scols))
    return caps, schedule, percore


def build_kernel(plan1, plan2):
    caps1, _ = plan1
    caps2, sched2, _ = plan2
    nc = bacc.Bacc(None, target_bir_lowering=False, debug=False)

    tot1 = int(caps1.sum())
    tot2 = int(caps2.sum())
    ncol2 = sum(len(s) for s in sched2)

    sbtot2 = np.zeros(NPARTS * NSB2, np.int64)
    for s in sched2:
        for _, ps in s:
            sbtot2[ps] += 1

    ftab = nc.dram_tensor("ftab", [N0, E1U], u64, kind="ExternalInput")
    idx1 = nc.dram_tensor("idx1", [128, tot1 // 16], i16, kind="ExternalInput")
    pos1 = nc.dram_tensor("pos1", [128, tot1 // 16], i16, kind="ExternalInput")
    iotab = nc.dram_tensor("iotab", [128, MAXIDX // 16], i16, kind="ExternalInput")
    idx2 = nc.dram_tensor("idx2", [128, tot2 // 16], i16, kind="ExternalInput")
    s2m = nc.dram_tensor("s2m", [128, ncol2 * 128], f8, kind="ExternalInput")
    ctabs = [
        nc.dram_tensor(f"ctab{g}", [CROWS, E1U], u64, kind="ExternalInput")
        for g in range(NG1)
    ]
    w1t = nc.dram_tensor("w1t", [128, 4, NH], f16, kind="ExternalInput")  # W1/16
    b1v = nc.dram_tensor("b1v", [128, 1], f32, kind="ExternalInput")
    w2t = nc.dram_tensor("w2t", [128, 2, NCLS], f16, kind="ExternalInput")  # W2/16
    scon = nc.dram_tensor("scon", [128, 8], f16, kind="ExternalInput")
    partial = nc.dram_tensor(
        "partial", [NPARTS, SG2, 128, 5 * NCLS], f32, kind="ExternalOutput"
    )

    # stage-1 per-group idx stream column offsets (in refs)
    cell_off = np.zeros(NG1 * NWIN + 1, np.int64)
    np.cumsum(caps1, out=cell_off[1:])

    with TileContext(nc) as tc:
        with (
            tc.tile_pool(name="const", bufs=1) as cpool,
            tc.tile_pool(name="ga", bufs=2) as gapool,
            tc.tile_pool(name="gb", bufs=4) as gbpool,
            tc.tile_pool(name="g2", bufs=6) as g2pool,
            tc.tile_pool(name="s2", bufs=6) as s2pool,
            tc.tile_pool(name="epi", bufs=3) as epool,
            tc.tile_pool(name="m0psum", bufs=2, space="PSUM") as mpool,
            tc.tile_pool(name="epipsum", bufs=3, space="PSUM") as eppool,
            tc.tile_pool(name="dram", bufs=1, space="DRAM") as dpool,
        ):
            # per-group idx/pos loads so the first gather starts early
            idx1_t = cpool.tile([128, tot1 // 16], i16)
            pos1_t = cpool.tile([128, tot1 // 16], i16)
            for g in range(NG1):
                a = int(cell_off[g * NWIN]) // 16
                b = int(cell_off[(g + 1) * NWIN]) // 16
                nc.sync.dma_start(idx1_t[:, a:b], idx1[:, a:b])
                nc.sync.dma_start(pos1_t[:, a:b], pos1[:, a:b])
            iotab_t = cpool.tile([128, MAXIDX // 16], i16)
            nc.sync.dma_start(iotab_t[:], iotab[:])
            idx2_t = cpool.tile([128, tot2 // 16], i16)
            nc.sync.dma_start(idx2_t[:], idx2[:])
            w1_t = cpool.tile([128, 4, NH], f16)
            nc.sync.dma_start(w1_t[:], w1t[:])
            b1_t = cpool.tile([128, 1], f32)
            nc.sync.dma_start(b1_t[:], b1v[:])
            w2_t = cpool.tile([128, 2, NCLS], f16)
            nc.sync.dma_start(w2_t[:], w2t[:])
            scon_t = cpool.tile([128, 8], f16)
            nc.sync.dma_start(scon_t[:], scon[:])

            qtabs = [dpool.tile([QPART_ROWS[p], E2U], u64, name=f"qtab{p}")
                     for p in range(NPARTS)]

            s2_tiles = []
            off2 = 0
            for i in range(NPARTS * SG2):
                ncell = len(sched2[i])
                if ncell:
                    t = cpool.tile([128, ncell, 128], f8, name=f"s_t{i}")
                    nc.sync.dma_start(t[:], s2m[:, off2 * 128 : (off2 + ncell) * 128])
                    s2_tiles.append(t)
                else:
                    s2_tiles.append(None)
                off2 += ncell

            # ---- stage 1 ----
            def do_gathers(g):
                tiles = []
                for w in range(NWIN):
                    cell = g * NWIN + w
                    n = int(caps1[cell])
                    if n == 0:
                        tiles.append(None)
                        continue
                    nb = _rup(n, 128) // 128
                    g_t = gapool.tile([128, nb, E1U], u64, tag="ga", name=f"g_t{w}")
                    wsz = min(WINDOW, N0 - w * WINDOW)
                    base = cell_off[cell]
                    off = 0
                    while off < n:
                        m = min(MAXIDX, n - off)
                        mb = _rup(m, 128) // 128
                        nc.gpsimd.dma_gather(
                            out_ap=g_t[:, off // 128 : off // 128 + mb, :],
                            in_ap=ftab[w * WINDOW : w * WINDOW + wsz, :],
                            idxs_ap=idx1_t[:, (base + off) // 16 : (base + off + m) // 16],
                            num_idxs=m,
                            num_idxs_reg=m,
                            elem_size=E1U,
                        )
                        off += m
                    tiles.append(g_t)
                return tiles

            def do_scatters(g, tiles):
                ctab = ctabs[g]
                for w in range(NWIN):
                    cell = g * NWIN + w
                    n = int(caps1[cell])
                    if n == 0:
                        continue
                    g_t = tiles[w]
                    base = cell_off[cell]
                    off = 0
                    while off < n:
                        m = min(MAXIDX, n - off)
                        mb = _rup(m, 128) // 128
                        nc.gpsimd.dma_scatter_add(
                            out_ap=ctab[:],
                            in_ap=g_t[:, off // 128 : off // 128 + mb, :],
                            idxs_ap=pos1_t[:, (base + off) // 16 : (base + off + m) // 16],
                            num_idxs=m,
                            num_idxs_reg=m,
                            elem_size=E1U,
                        )
                        off += m

            def do_compute(g):
                ctab = ctabs[g]
                for j in range(GROUP1):
                    sb = g * GROUP1 + j
                    b_t = gbpool.tile([128, 16, E1U], u64, tag="gb", name="b_t")
                    for h in range(2):
                        nc.gpsimd.dma_gather(
                            out_ap=b_t[:, h * 8 : h * 8 + 8, :],
                            in_ap=ctab[j * 2048 + h * 1024 : j * 2048 + h * 1024 + 1024, :],
                            idxs_ap=iotab_t[:],
                            num_idxs=MAXIDX,
                            num_idxs_reg=MAXIDX,
                            elem_size=E1U,
                        )
                    m0t = mpool.tile([128, 4, 128], f32, tag="m0", name="m0t")
                    bf8 = b_t[:].bitcast(f8)  # [128, 16, 512]
                    for b in range(16):
                        for k in range(4):
                            nc.tensor.matmul(
                                out=m0t[:, k, 8 * b : 8 * b + 8],
                                lhsT=bf8[:, b, k * 128 : (k + 1) * 128],
                                rhs=scon_t[:],
                                start=True,
                                stop=True,
                            )
                    # epilogue
                    m0s = epool.tile([128, 4, 128], f16, tag="m0s", name="m0s")
                    nc.vector.tensor_copy(m0s[:], m0t[:])
                    h1p = eppool.tile([128, 128], f32, tag="ep", name="h1p")
                    for k in range(4):
                        nc.tensor.matmul(
                            out=h1p[:],
                            lhsT=w1_t[:, k, :],
                            rhs=m0s[:, k, :],
                            start=(k == 0),
                            stop=(k == 3),
                        )
                    h1s = epool.tile([128, 128], f16, tag="h1", name="h1s")
                    rs = epool.tile([128, 128], f16, tag="r", name="rs")
                    nc.scalar.activation(
                        h1s[:], h1p[:], mybir.ActivationFunctionType.Identity, bias=b1_t[:, :1]
                    )
                    nc.scalar.activation(
                        rs[:], h1p[:], mybir.ActivationFunctionType.Relu, bias=b1_t[:, :1]
                    )
                    qp = eppool.tile([128, NCLS], f32, tag="ep", name="qp")
                    nc.tensor.matmul(out=qp[:], lhsT=h1s[:], rhs=w2_t[:, 0, :], start=True, stop=False)
                    nc.tensor.matmul(out=qp[:], lhsT=rs[:], rhs=w2_t[:, 1, :], start=False, stop=True)
                    qs = epool.tile([128, NCLS], f16, tag="qs", name="qs")
                    nc.scalar.activation(qs[:], qp[:], mybir.ActivationFunctionType.Copy)
                    part = next(p for p, (a2, b2) in enumerate(QPART_GROUPS) if a2 <= g < b2)
                    r0 = sb * 128 - QPART_BASE[part]
                    dst = qtabs[part][r0 : r0 + 128, :]
                    nc.sync.dma_start(dst.bitcast(f16)[:, :NCLS], qs[:])

            # ---- stage 2 ----
            cell2_off = np.zeros(NPARTS * SG2 + 1, np.int64)
            np.cumsum(caps2, out=cell2_off[1:])
            sb2_seq = np.zeros(NPARTS * NSB2, np.int64)
            pp = {}
            mcol = [0]

            def do_stage2_cell(i):
                qh = i // SG2
                qt = qtabs[qh]
                n = int(caps2[i])
                pgrp = epool.tile([128, 5, NCLS], f32, tag="pg", name="pg", bufs=4)
                cellpp = eppool.tile([128, 5, NCLS], f32, tag="ep", name="cellpp")
                written = set()
                if n:
                    nb = _rup(n, 128) // 128
                    g_t = g2pool.tile([128, nb, E2U], u64, tag="g2", name="g_t2")
                    base = cell2_off[i]
                    off = 0
                    while off < n:
                        m = min(MAXIDX, n - off)
                        mb = _rup(m, 128) // 128
                        nc.gpsimd.dma_gather(
                            out_ap=g_t[:, off // 128 : off // 128 + mb, :],
                            in_ap=qt[:],
                            idxs_ap=idx2_t[:, (base + off) // 16 : (base + off + m) // 16],
                            num_idxs=m,
                            num_idxs_reg=m,
                            elem_size=E2U,
                        )
                        off += m
                    gf16 = g_t[:].bitcast(f16)  # [128, nb, 128]
                    ncell = len(sched2[i])
                    s_t = s2_tiles[i]
                    for mi, (b, ps) in enumerate(sched2[i]):
                        jj = (ps % NSB2) % 5
                        nc.tensor.matmul(
                            out=cellpp[:, jj, :],
                            lhsT=s_t[:, mi, :],
                            rhs=gf16[:, b, :NCLS],
                            start=(sb2_seq[ps] == 0),
                            stop=(sb2_seq[ps] == sbtot2[ps] - 1),
                        )
                        sb2_seq[ps] += 1
                        if sb2_seq[ps] == sbtot2[ps]:
                            written.add(jj)
                    mcol[0] += ncell
                for jj in range(5):
                    if jj not in written:
                        nc.vector.memset(cellpp[:, jj, :], 0.0)
                if i % 2 == 0:
                    nc.vector.tensor_copy(pgrp[:], cellpp[:])
                else:
                    nc.scalar.activation(
                        pgrp[:], cellpp[:], mybir.ActivationFunctionType.Copy
                    )
                sg = i % SG2
                eng = nc.sync if i % 2 == 0 else nc.scalar
                eng.dma_start(
                    partial[qh, sg, :, :].rearrange("p (j e) -> p j e", j=5),
                    pgrp[:, :, :],
                )

            # software pipeline: gathers(g) | compute(g-1) | scatters(g);
            # stage-2 parts go last, in readiness order, so no Pool
            # instruction queues behind a long epilogue dependency
            tiles = do_gathers(0)
            do_scatters(0, tiles)
            for g in range(1, NG1):
                tiles = do_gathers(g)
                do_compute(g - 1)
                do_scatters(g, tiles)
                if g - 1 == 2:
                    for i in range(SG2):
                        do_stage2_cell(i)
            do_compute(NG1 - 1)
            for i in range(SG2, NPARTS * SG2):
                do_stage2_cell(i)
    nc.compile()
    return nc


def _host_inputs(features, src0, src1, W1, b1, W2):
    core_of, loc_of = _assign_nodes(src0)
    plan1 = _plan_stage1(src0, core_of, loc_of)
    plan2 = _plan_stage2(src1, core_of, loc_of)

    ftab_f8 = np.zeros((N0, E1), ml_dtypes.float8_e3m4)
    ftab_f8[:, :IN_F] = np.asarray(features, np.float32).astype(ml_dtypes.float8_e3m4)
    ftab_u64 = np.ascontiguousarray(ftab_f8).view(np.uint64)

    w1_np = np.zeros((128, 4, NH), np.float16)
    w1f = np.zeros((E1, NH), np.float32)
    w1f[:IN_F] = np.asarray(W1, np.float32) / FANOUT
    for k in range(4):
        w1_np[:, k, :] = w1f[k * 128 : (k + 1) * 128].astype(np.float16)
    b1_np = np.asarray(b1, np.float32).reshape(128, 1)
    w2_np = np.zeros((128, 2, NCLS), np.float16)
    w2f = np.asarray(W2, np.float32) / FANOUT
    w2_np[:, 0, :] = w2f[:NH].astype(np.float16)
    w2_np[:, 1, :] = w2f[NH:].astype(np.float16)
    scon_np = np.zeros((128, 8), np.float16)
    scon_np[np.arange(128), np.arange(128) // 16] = 1.0
    iotab_np = _wrap_idxs(np.arange(MAXIDX, dtype=np.int16))
    czero = np.zeros((CROWS, E1U), np.uint64)

    in_maps = []
    for c in range(NCORES):
        idx1c, pos1c = plan1[1][c]
        idx2c, s2c = plan2[2][c]
        im = {
            "ftab": ftab_u64,
            "idx1": np.ascontiguousarray(_wrap_idxs(idx1c)),
            "pos1": np.ascontiguousarray(_wrap_idxs(pos1c)),
            "iotab": iotab_np,
            "idx2": np.ascontiguousarray(_wrap_idxs(idx2c)),
            "s2m": np.ascontiguousarray(s2c),
            "w1t": w1_np,
            "b1v": b1_np,
            "w2t": w2_np,
            "scon": scon_np,
        }
        for g in range(NG1):
            im[f"ctab{g}"] = czero
        in_maps.append(im)
    return plan1, plan2, in_maps


_cache = {}


def kernel(features, src0, src1, W1, b1, W2, b2):
    plan1, plan2, in_maps = _host_inputs(features, src0, src1, W1, b1, W2)
    key = hashlib.sha256(
        b"|".join(
            [plan1[0].tobytes(), plan2[0].tobytes(), str(plan2[1]).encode(), b"v2"]
        )
    ).hexdigest()
    if key not in _cache:
        _cache[key] = build_kernel(plan1, plan2)
    nc = _cache[key]
    res = run_bass_kernel_spmd(nc, in_maps, core_ids=list(range(NCORES)))
    out = np.zeros((SEEDS, NCLS), np.float64)
    for c in range(NCORES):
        p = res.results[c]["partial"].astype(np.float64)
        p = p.reshape(NPARTS, SG2, 128, 5, NCLS).transpose(0, 1, 3, 2, 4)
        out += p.reshape(NPARTS, SG2 * 640, NCLS)[:, :SEEDS].sum(axis=0)
    out = out + np.asarray(b2, np.float64)[None, :]
    return out.astype(np.float32)


if __name__ == "__main__":
    rng = np.random.default_rng(0)
    feats = rng.standard_normal((N0, IN_F), dtype=np.float32)
    src0 = rng.integers(0, N0, size=(N1, FANOUT))
    src1 = rng.integers(0, N1, size=(N2, FANOUT))
    W1 = rng.standard_normal((IN_F, NH), dtype=np.float32) * 0.05
    b1 = np.zeros(NH, np.float32)
    W2 = rng.standard_normal((2 * NH, NCLS), dtype=np.float32) * 0.05
    b2 = np.zeros(NCLS, np.float32)
    out = kernel(feats, src0, src1, W1, b1, W2, b2)
    m0 = feats[src0].mean(axis=1)
    h1 = m0 @ W1 + b1
    h1 = np.concatenate([h1, np.maximum(h1, 0)], axis=1)
    m1 = h1[src1].mean(axis=1)
    ref = m1 @ W2 + b2
    rel = np.abs(out - ref) / (np.abs(ref) + 1e-5)
    print("max rel err:", rel.max(), "mean:", rel.mean())
    print("norm rel:", np.linalg.norm(out - ref) / np.linalg.norm(ref))
